# revision 42
# baseline (speedup 1.0000x reference)
"""Multi-head attention TRN2 kernel (8 NeuronCores, SPMD) — fp8/bf16 rewrite.

Sharding: data parallel over batch (4) x tensor parallel over head halves
(2 groups of 8 heads) = 8 shards.

Per-core pipeline:
  xt  = x^T via DMA XBAR transpose (bf16)                 [128, 8, 2048]
  Q^T = wq^T @ xt + bq  (bf16 matmul) -> qt fp8e4         [128 dq, 4x(512+512z)]
  K^T = wk^T @ xt       (bf16 matmul) -> kt fp8e4         [128 dq, 2048+128z]
  V   = xt^T-chunks @ wv (bf16)       -> vt fp8e4 (+ones) [128, 16*520]
  S^T = K_h^T Q_h  fp8 DoubleRow, zero 2nd plane          [128 kpos, 512 q]
  P   = exp(0.125 S): ACT exact exp->fp8, or DVE one-pass
        Schraudolph (int8 bitcast of fp8e4)               [128, 2x512]
  ctx'^T = [V|1]^T P  fp8 DoubleRow, planes = kt pair     [65, 512] psum
  corr: host ships exact col-sums of V; core computes 1^T V8 and adds
        (vsumE - vsum8) per ctx row during the cx copy (fixes the
        V-quantization bias through softmax near-uniformity)
  y_h = wf_h^T cx  (fp16)                                 [64, 512]
Host combines: out_b = sum_h (y_h / sums_h).T + bf.
"""

import json
import math
import os
import sys
import types

import numpy as np
import ml_dtypes

# ---------------------------------------------------------------------------
# Environment shims (walrus sync-wait limit + optional NTFF profile hook)
# ---------------------------------------------------------------------------

_patched = False


def _ensure_patches():
    global _patched
    if _patched:
        return
    import concourse.bass_utils as bass_utils
    import concourse.bass2jax as bass2jax
    import concourse.tile as tile
    from concourse.vector_clock import ScopedClock

    MAX_WAITS = 1
    MARK = "__waits_split__"

    def _split(bir_json: bytes) -> bytes:
        d = json.loads(bir_json)
        if d.get(MARK):
            return bir_json
        n_new = 0
        for fn in d.get("functions", []):
            for bb in fn.get("blocks", []):
                insts = bb.get("instructions", [])
                out = []
                for inst in insts:
                    si = inst.get("sync_info")
                    waits = (si or {}).get("on_wait") or []
                    if len(waits) > MAX_WAITS:
                        extra = waits[:-MAX_WAITS]
                        si["on_wait"] = waits[-MAX_WAITS:]
                        for k in range(0, len(extra), MAX_WAITS):
                            out.append({
                                "name": f"WSP-{n_new}",
                                "opcode": "NoOp",
                                "engine": inst["engine"],
                                "ins": [],
                                "outs": [],
                                "text_hint": "wait_split",
                                "sync_info": {
                                    "on_wait": extra[k:k + MAX_WAITS],
                                    "on_update": [],
                                },
                            })
                            n_new += 1
                    out.append(inst)
                if len(out) != len(insts):
                    bb["instructions"] = out
        d[MARK] = True
        return json.dumps(d).encode()

    orig_compile = bass_utils.compile_bir_kernel

    def patched_compile(bir_json, tmpdir, neff_name="file.neff"):
        return orig_compile(_split(bir_json), tmpdir, neff_name)

    bass_utils.compile_bir_kernel = patched_compile
    if getattr(bass2jax, "compile_bir_kernel", None) is not None:
        bass2jax.compile_bir_kernel = patched_compile

    def _drain_and_barrier(self, tick_clock, wait_clock):
        nc = self.nc
        probe = nc.sync.nop(nofuse=True, hint="drain_waits_probe")
        wait_clock.add_sem_waits(
            probe.ins, ScopedClock({None: tick_clock.global_clock})
        )
        nc.sync.drain()
        nc.all_engine_barrier()
        assert self.sems is not None
        popped = nc._tile_sem_poison_stack.pop()
        assert popped is self._sem_poison
        nc.clear_and_free_semaphores(list(self.sems.allocated().values()))
        nc.all_engine_barrier()

    tile.TileContext._drain_and_barrier = _drain_and_barrier
    _patched = True


def _ensure_profile_hook():
    try:
        import antenv
    except ImportError:
        return
    if "antenv.axon_hooks" not in sys.modules:
        m = types.ModuleType("antenv.axon_hooks")
        m._hook = None
        m.set_axon_ntff_profile_hook = lambda h: setattr(m, "_hook", h)
        m.get_axon_ntff_profile_hook = lambda: m._hook
        sys.modules["antenv.axon_hooks"] = m
        antenv.axon_hooks = m
    mod = sys.modules["antenv.axon_hooks"]
    if mod.get_axon_ntff_profile_hook() is None:
        try:
            from trn_agent_boot.trn_boot import _ntff_profile_via_ctypes
            mod.set_axon_ntff_profile_hook(
                _ntff_profile_via_ctypes("/opt/axon/libaxon_pjrt.so")
            )
        except Exception:
            pass


# ---------------------------------------------------------------------------
# Problem constants (hardcoded per contract)
# ---------------------------------------------------------------------------

B, S, DIN = 4, 2048, 1024
H, D = 16, 64
PROJ = H * D          # 1024
NCORES = 8
PL = PROJ // 2        # 512 per-core projection (8 heads)
HL = 8                # local heads
NPAIR = 4             # local head pairs
ST = S // 128         # 16 seq tiles (kpos chunks)
KT = DIN // 128       # 8 contraction tiles
VBLK = 256            # per (ktpair, head): 2 planes x [V(64)|ones|zeros(63)]
PAIRBLK = HL * VBLK   # 2048 cols per ktpair

# exp engine split: of every 16 chunks, this many go to ACT (exact exp),
# the rest to DVE (one-pass Schraudolph into fp8e4 bit patterns).
ACT_SHARE16 = int(os.environ.get("MHA_ACT_SHARE16", "8"))
SCHRAUD_C = float(os.environ.get("MHA_SCHRAUD_C", "-0.35"))
LOG2E = 1.4426950408889634

_cache = {}


def _build_program():
    import concourse.bass as bass
    import concourse.mybir as mybir
    import concourse.tile as tile

    f32 = mybir.dt.float32
    bf16 = mybir.dt.bfloat16
    f16 = mybir.dt.float16
    f8 = mybir.dt.float8e4
    i8 = mybir.dt.int8
    EXP = mybir.ActivationFunctionType.Exp
    IDENT = mybir.ActivationFunctionType.Identity
    DR = mybir.MatmulPerfMode.DoubleRow
    ADD = mybir.AluOpType.add
    SUB = mybir.AluOpType.subtract
    MULT = mybir.AluOpType.mult

    nc = bass.Bass("TRN2", target_bir_lowering=False, debug=False)

    x_d = nc.dram_tensor("x", [S, DIN], f16, kind="ExternalInput")
    wq_d = nc.dram_tensor("wq", [128, NPAIR * KT * 128], f16, kind="ExternalInput")
    wk_d = nc.dram_tensor("wk", [128, NPAIR * KT * 128], f16, kind="ExternalInput")
    wv_d = nc.dram_tensor("wv", [128, KT * PL], f16, kind="ExternalInput")
    bq_d = nc.dram_tensor("bq", [PL], f32, kind="ExternalInput")
    bvb_d = nc.dram_tensor("bvb", [128, PL], f16, kind="ExternalInput")
    wf_d = nc.dram_tensor("wf", [65, PL], f16, kind="ExternalInput")
    vse_d = nc.dram_tensor("vse", [64, HL], f32, kind="ExternalInput")
    vtz_d = nc.dram_tensor("vtz", [128, (ST // 2) * PAIRBLK], f8, kind="ExternalInput")
    y_d = nc.dram_tensor("y", [HL, D, S], f16, kind="ExternalOutput")
    s_d = nc.dram_tensor("s", [HL, S], f16, kind="ExternalOutput")

    with tile.TileContext(nc) as tc:
        with (
            tc.tile_pool(name="big", bufs=1) as big,
            tc.tile_pool(name="qk", bufs=2) as qkpool,
            tc.tile_pool(name="wblk", bufs=6) as wblk,
            tc.tile_pool(name="pt", bufs=6) as ptpool,
            tc.tile_pool(name="cx", bufs=2) as cxpool,
            tc.tile_pool(name="yst", bufs=2) as yst,
            tc.tile_pool(name="ps", bufs=1, space="PSUM") as ps,
            tc.tile_pool(name="ps2", bufs=4, space="PSUM") as ps2,
        ):
            # ---- persistent SBUF ------------------------------------------
            xt = big.tile([128, KT * S], f16, tag="xt")       # x^T, kt-blocked
            wv_sb = big.tile([128, KT * PL], f16, tag="wv")   # resident wv
            wq_sb = big.tile([128, NPAIR * KT * 128], f16, tag="wqs")
            wk_sb = big.tile([128, NPAIR * KT * 128], f16, tag="wks")
            vt = big.tile([128, (ST // 2) * PAIRBLK], f8, tag="vt")
            bqt = big.tile([128, NPAIR], f32, tag="bqt")
            bvt = big.tile([128, PL], f16, tag="bvt")
            wft = big.tile([65, PL], f16, tag="wft")
            vse = big.tile([64, HL], f32, tag="vse")
            ones8 = big.tile([128, 1], f8, tag="ones8")
            ident = big.tile([1, 1], f32, tag="ident")
            corrT = big.tile([65, HL], f32, tag="corrT")
            vs_sb = big.tile([1, PL], f32, tag="vs_sb")

            # Priority DMAs first: pair-0 weights + x^T transposes, so the
            # first projection chains start within a few us. The rest
            # streams in under the upfront compute block.
            sl0 = slice(0, KT * 128)
            nc.sync.dma_start(wq_sb[:, sl0], wq_d[:, sl0])
            nc.sync.dma_start(wk_sb[:, sl0], wk_d[:, sl0])
            nc.sync.dma_start(bqt[:], bq_d[:].rearrange("(t p) -> p t", p=128))
            for c in range(KT):
                nc.sync.dma_start_transpose(
                    xt[:, c * S:(c + 1) * S], x_d[:, c * 128:(c + 1) * 128]
                )
            nc.sync.dma_start(wv_sb[:], wv_d[:])
            nc.sync.dma_start(bvt[:], bvb_d[:])
            for c in range(1, 4):
                sl_ = slice(c * 1024, (c + 1) * 1024)
                nc.sync.dma_start(wq_sb[:, sl_], wq_d[:, sl_])
                nc.sync.dma_start(wk_sb[:, sl_], wk_d[:, sl_])
            nc.sync.dma_start(wft[:], wf_d[:])
            nc.sync.dma_start(vse[:], vse_d[:])

            nc.vector.memset(ones8[:], 1.0)
            nc.vector.memset(ident[:], 1.0)
            # vt ones/zeros scaffold pre-built on host
            nc.sync.dma_start(vt[:], vtz_d[:])

            # PSUM: sA/sB (ps2, [128,1024] x2bufs = 4 banks), c0 c1 (ctx),
            # p0 p1 (proj/fc/misc) = 8 banks total.

            def qt_tile():
                # [128 dq, 4 x (512 data + 512 zeros)] fp8
                return qkpool.tile([128, 2048], f16, tag="qt", name="qt")

            def kt_tile():
                # [128 dq, 2048 data + 128 zeros] fp8
                return qkpool.tile([128, 2048], f16, tag="ktr", name="ktr")

            # ---- QK projection (fp16, resident weights) ------------------
            def emit_qk_quanta(p, use_s=False):
                """Half-chain quanta (4 matmuls each; copy on 2nd half)."""
                qt_p = qt_tile()
                kt_p = kt_tile()
                quanta = []
                for w_sb, dst, is_q in ((wq_sb, qt_p, True), (wk_sb, kt_p, False)):
                    for ntg in range(2):
                        for i in range(2):
                            nt = ntg * 2 + i
                            for half in range(2):
                                def quantum(w_sb=w_sb, dst=dst, is_q=is_q,
                                            nt=nt, i=i, half=half):
                                    acc = ps.tile([128, 512], f32,
                                                  tag=f"p{i}", name="acc")
                                    for kk in range(4):
                                        kt = half * 4 + kk
                                        wof = (p * KT + kt) * 128
                                        nc.tensor.matmul(
                                            acc[:],
                                            w_sb[:, wof:wof + 128],
                                            xt[:, kt * S + nt * 512:
                                               kt * S + (nt + 1) * 512],
                                            start=(kt == 0), stop=(kt == KT - 1),
                                        )
                                    if half == 1:
                                        if is_q:
                                            nc.scalar.activation(
                                                dst[:, nt * 512:(nt + 1) * 512],
                                                acc[:], IDENT,
                                                bias=bqt[:, p:p + 1],
                                            )
                                        else:
                                            nc.scalar.copy(
                                                dst[:, nt * 512:(nt + 1) * 512],
                                                acc[:])
                                quanta.append(quantum)
                return (qt_p, kt_p), quanta

            # ---- V projection (bf16) -> vt fp8 ----------------------------
            def v_quantum(st):
                def quantum():
                    acc = ps.tile([128, PL], f32, tag="p0", name="acc")
                    for kt in range(KT):
                        nc.tensor.matmul(
                            acc[:],
                            xt[:, kt * S + st * 128: kt * S + (st + 1) * 128],
                            wv_sb[:, kt * PL:(kt + 1) * PL],
                            start=(kt == 0), stop=(kt == KT - 1),
                        )
                    t, i = st // 2, st % 2
                    dstv = vt[:, t * PAIRBLK:(t + 1) * PAIRBLK].rearrange(
                        "p (h a x) -> p h a x", a=2, x=128)[:, :, i, 0:64]
                    nc.vector.tensor_tensor(dstv, acc[:], bvt[:], op=ADD)
                return quantum

            # ---- V-sum correction chain -----------------------------------
            def vsum_quantum():
                def quantum():
                    vsp = ps.tile([1, PL], f32, tag="p0", name="vsp")
                    for st in range(ST):
                        t, i = st // 2, st % 2
                        mv = vt[:, t * PAIRBLK:(t + 1) * PAIRBLK].rearrange(
                            "p (h a x) -> p h a x", a=2, x=128)[:, :, i, 0:64]
                        nc.tensor.matmul(
                            vsp[:], ones8[:], mv,
                            start=(st == 0), stop=(st == ST - 1),
                        )
                    nc.vector.tensor_copy(vs_sb[:], vsp[:])
                return quantum

            def corr_quantum():
                def quantum():
                    v8t = ps.tile([64, HL], f32, tag="p0", name="v8t")
                    for h in range(HL):
                        nc.tensor.transpose(
                            v8t[:, h:h + 1],
                            vs_sb[0:1, h * 64:(h + 1) * 64],
                            ident[:],
                        )
                    nc.vector.tensor_tensor(
                        corrT[0:64, :], vse[:], v8t[:], op=SUB)
                    nc.vector.memset(corrT[64:65, :], 0.0)
                return quantum

            # ---- attention ------------------------------------------------
            chunk_no = [0]

            def emit_attention(p, qc, qt_p, kt_p, fill):
                """One (pair, qchunk): 2 heads x 8 ktpairs, then 2 FCs.

                ctx matmuls lag the scores by one kt-pair so the exp result
                they consume is always ready when the PE reaches them."""
                q0 = qc * 512
                ctxs = [ps.tile([128, 512], f32, tag="c0", name="ctx"),
                        ps.tile([128, 512], f32, tag="c1", name="ctx")]

                def do_ctx(h, t, pt_t):
                    gh = p * 2 + h
                    nc.tensor.matmul(
                        ctxs[h][:],
                        vt[:, t * PAIRBLK + gh * VBLK:
                           t * PAIRBLK + (gh + 1) * VBLK].rearrange(
                            "p (a x) -> p a x", x=128),
                        pt_t[:].rearrange("p (a x) -> p a x", a=2),
                        start=(t == 0), stop=(t == ST // 2 - 1),
                        perf_mode=DR,
                    )

                pend = []  # [(h, t, pt_t)] awaiting ctx matmuls
                for t in range(ST // 2):  # 8 kt pairs
                    pts = [ptpool.tile([128, 1024], f8, tag="pt", name="pt")
                           for _ in range(2)]
                    for i in range(2):
                        ki = 2 * t + i
                        # adjacent disjoint-row-group matmuls run
                        # concurrently in the PE array
                        for h in range(2):
                            r0 = h * 64
                            s_ps = ps2.tile([128, 512], f32, tag="s",
                                            name="s_ps")
                            nc.tensor.matmul(
                                s_ps[:],
                                kt_p[r0:r0 + 64, ki * 128:(ki + 1) * 128],
                                qt_p[r0:r0 + 64, q0:q0 + 512],
                                start=True, stop=True,
                                tile_position=(r0, 0),
                            )
                            cn = chunk_no[0]
                            chunk_no[0] += 1
                            # Bresenham interleave: ACT/DVE alternate
                            if ((cn + 1) * ACT_SHARE16) // 16 > \
                                    (cn * ACT_SHARE16) // 16:
                                nc.scalar.activation(
                                    pts[h][:, i * 512:(i + 1) * 512],
                                    s_ps[:], EXP, scale=0.125)
                            else:
                                nc.vector.tensor_scalar(
                                    pts[h][:, i * 512:(i + 1) * 512].bitcast(i8),
                                    s_ps[:],
                                    0.125 * 8.0 * LOG2E, 56.0 + SCHRAUD_C,
                                    op0=MULT, op1=ADD,
                                )
                            if fill and cn % 4 == 2:
                                fill.pop(0)()
                    while pend:
                        do_ctx(*pend.pop(0))
                    pend = [(0, t, pts[0]), (1, t, pts[1])]
                while pend:
                    do_ctx(*pend.pop(0))

                for h in range(2):
                    gh = p * 2 + h
                    ctx = ctxs[h]
                    cx = cxpool.tile([65, 512], f16, tag="cx", name="cx")
                    nc.scalar.activation(
                        cx[:], ctx[0:65, :], IDENT,
                        bias=corrT[:, gh:gh + 1])
                    nc.sync.dma_start(s_d[gh, qc * 512:(qc + 1) * 512],
                                      cx[64:65, :])
                    yp = ps.tile([64, 512], f32, tag=f"c{h}", name="yp")
                    nc.tensor.matmul(
                        yp[:], wft[:, gh * 64:(gh + 1) * 64], cx[:],
                        start=True, stop=True,
                    )
                    yo = yst.tile([64, 512], f16, tag="yo", name="yo")
                    nc.vector.tensor_copy(yo[:], yp[:])
                    nc.sync.dma_start(
                        y_d[gh, :, qc * 512:(qc + 1) * 512], yo[:])

            # ---- schedule -------------------------------------------------
            # upfront (dense PE block, warms the clock): qk pair 0, all of
            # V, the V-sum correction chain
            (qk_cur, quanta0) = emit_qk_quanta(0)
            for fn in quanta0:
                fn()
            for st in range(ST):
                v_quantum(st)()
            vsum_quantum()()
            corr_quantum()()

            fill = []
            qk_next = None
            for p in range(NPAIR):
                if p > 0:
                    qk_cur = qk_next
                for qc in range(4):
                    if qc == 0 and p + 1 < NPAIR:
                        (qk_next, quanta) = emit_qk_quanta(p + 1)
                        fill.extend(quanta)
                    emit_attention(p, qc, *qk_cur, fill)
            while fill:
                fill.pop(0)()

    return nc


def _prepare_in_maps(x, Wq, bq, Wk, bk, Wv, bv, Wf, bf):
    f16 = np.float16
    f8 = ml_dtypes.float8_e4m3
    in_maps = []
    x_16 = [np.ascontiguousarray(x[b]).astype(f16) for b in range(B)]

    # vt scaffold: zeros with 1.0 at col 64 of every 128-block
    vtz = np.zeros((128, (ST // 2) * PAIRBLK), dtype=f8)
    vtz.reshape(128, -1, 128)[:, :, 64] = f8(1.0)

    for core in range(NCORES):
        b, g = core // 2, core % 2
        sl = slice(g * PL, (g + 1) * PL)

        def _tile_w(w):  # [1024, 512] -> [128, (pair, kt, 128)]
            return np.ascontiguousarray(
                w.reshape(KT, 128, NPAIR, 128).transpose(1, 2, 0, 3).reshape(
                    128, NPAIR * KT * 128)
            ).astype(f16)

        wv_g = np.ascontiguousarray(Wv[:, sl]).astype(f16)
        # exact col-sums of V (incl bias) for the correction, [64, HL]
        v_exact = x_16[b].astype(np.float32) @ wv_g.astype(np.float32) \
            + bv[sl].astype(np.float32)
        vse = v_exact.sum(axis=0, dtype=np.float64).astype(np.float32)
        vse = np.ascontiguousarray(vse.reshape(HL, 64).T)  # [64, HL]

        wf_s = np.zeros((65, PL), dtype=np.float16)
        for h in range(HL):
            wf_s[0:64, h * 64:(h + 1) * 64] = \
                Wf[g * PL + h * 64: g * PL + (h + 1) * 64, :]

        in_maps.append({
            "x": x_16[b],
            "wq": _tile_w(Wq[:, sl]),
            "wk": _tile_w(Wk[:, sl]),
            "wv": np.ascontiguousarray(
                wv_g.reshape(KT, 128, PL).transpose(1, 0, 2).reshape(
                    128, KT * PL)),
            "bq": np.ascontiguousarray(bq[sl]).astype(np.float32),
            "bvb": np.broadcast_to(bv[sl], (128, PL)).astype(f16).copy(),
            "wf": wf_s,
            "vse": vse,
            "vtz": vtz,
        })
    return in_maps


def kernel(**inputs):
    _ensure_patches()
    _ensure_profile_hook()
    from concourse.bass_utils import run_bass_kernel_spmd

    if "nc" not in _cache:
        _cache["nc"] = _build_program()
    nc = _cache["nc"]

    inp = {k: np.asarray(v, dtype=np.float32) for k, v in inputs.items()}
    in_maps = _prepare_in_maps(**inp)

    trace = bool(os.environ.get("MHA_TRACE"))
    res = run_bass_kernel_spmd(nc, in_maps, list(range(NCORES)), trace=trace)
    _cache["last_results"] = res

    bf = inp["bf"]
    out = np.empty((B, S, D), dtype=np.float32)
    for b in range(B):
        acc = np.zeros((D, S), dtype=np.float64)
        for core in (2 * b, 2 * b + 1):
            yc = np.asarray(res.results[core]["y"]).astype(np.float64)
            sc = np.asarray(res.results[core]["s"]).astype(np.float64)
            acc += (yc / sc[:, None, :]).sum(axis=0)
        out[b] = acc.T + bf
    return out


# revision 43
# speedup vs baseline: 1.0227x; 1.0227x over previous
"""Multi-head attention TRN2 kernel (8 NeuronCores, SPMD) — fp8/bf16 rewrite.

Sharding: data parallel over batch (4) x tensor parallel over head halves
(2 groups of 8 heads) = 8 shards.

Per-core pipeline:
  xt  = x^T via DMA XBAR transpose (bf16)                 [128, 8, 2048]
  Q^T = wq^T @ xt + bq  (bf16 matmul) -> qt fp8e4         [128 dq, 4x(512+512z)]
  K^T = wk^T @ xt       (bf16 matmul) -> kt fp8e4         [128 dq, 2048+128z]
  V   = xt^T-chunks @ wv (bf16)       -> vt fp8e4 (+ones) [128, 16*520]
  S^T = K_h^T Q_h  fp8 DoubleRow, zero 2nd plane          [128 kpos, 512 q]
  P   = exp(0.125 S): ACT exact exp->fp8, or DVE one-pass
        Schraudolph (int8 bitcast of fp8e4)               [128, 2x512]
  ctx'^T = [V|1]^T P  fp8 DoubleRow, planes = kt pair     [65, 512] psum
  corr: host ships exact col-sums of V; core computes 1^T V8 and adds
        (vsumE - vsum8) per ctx row during the cx copy (fixes the
        V-quantization bias through softmax near-uniformity)
  y_h = wf_h^T cx  (fp16)                                 [64, 512]
Host combines: out_b = sum_h (y_h / sums_h).T + bf.
"""

import json
import math
import os
import sys
import types

import numpy as np
import ml_dtypes

# ---------------------------------------------------------------------------
# Environment shims (walrus sync-wait limit + optional NTFF profile hook)
# ---------------------------------------------------------------------------

_patched = False


def _ensure_patches():
    global _patched
    if _patched:
        return
    import concourse.bass_utils as bass_utils
    import concourse.bass2jax as bass2jax
    import concourse.tile as tile
    from concourse.vector_clock import ScopedClock

    MAX_WAITS = 1
    MARK = "__waits_split__"

    def _split(bir_json: bytes) -> bytes:
        d = json.loads(bir_json)
        if d.get(MARK):
            return bir_json
        n_new = 0
        for fn in d.get("functions", []):
            for bb in fn.get("blocks", []):
                insts = bb.get("instructions", [])
                out = []
                for inst in insts:
                    si = inst.get("sync_info")
                    waits = (si or {}).get("on_wait") or []
                    if len(waits) > MAX_WAITS:
                        extra = waits[:-MAX_WAITS]
                        si["on_wait"] = waits[-MAX_WAITS:]
                        for k in range(0, len(extra), MAX_WAITS):
                            out.append({
                                "name": f"WSP-{n_new}",
                                "opcode": "NoOp",
                                "engine": inst["engine"],
                                "ins": [],
                                "outs": [],
                                "text_hint": "wait_split",
                                "sync_info": {
                                    "on_wait": extra[k:k + MAX_WAITS],
                                    "on_update": [],
                                },
                            })
                            n_new += 1
                    out.append(inst)
                if len(out) != len(insts):
                    bb["instructions"] = out
        d[MARK] = True
        return json.dumps(d).encode()

    orig_compile = bass_utils.compile_bir_kernel

    def patched_compile(bir_json, tmpdir, neff_name="file.neff"):
        return orig_compile(_split(bir_json), tmpdir, neff_name)

    bass_utils.compile_bir_kernel = patched_compile
    if getattr(bass2jax, "compile_bir_kernel", None) is not None:
        bass2jax.compile_bir_kernel = patched_compile

    def _drain_and_barrier(self, tick_clock, wait_clock):
        nc = self.nc
        probe = nc.sync.nop(nofuse=True, hint="drain_waits_probe")
        wait_clock.add_sem_waits(
            probe.ins, ScopedClock({None: tick_clock.global_clock})
        )
        nc.sync.drain()
        nc.all_engine_barrier()
        assert self.sems is not None
        popped = nc._tile_sem_poison_stack.pop()
        assert popped is self._sem_poison
        nc.clear_and_free_semaphores(list(self.sems.allocated().values()))
        nc.all_engine_barrier()

    tile.TileContext._drain_and_barrier = _drain_and_barrier
    _patched = True


def _ensure_profile_hook():
    try:
        import antenv
    except ImportError:
        return
    if "antenv.axon_hooks" not in sys.modules:
        m = types.ModuleType("antenv.axon_hooks")
        m._hook = None
        m.set_axon_ntff_profile_hook = lambda h: setattr(m, "_hook", h)
        m.get_axon_ntff_profile_hook = lambda: m._hook
        sys.modules["antenv.axon_hooks"] = m
        antenv.axon_hooks = m
    mod = sys.modules["antenv.axon_hooks"]
    if mod.get_axon_ntff_profile_hook() is None:
        try:
            from trn_agent_boot.trn_boot import _ntff_profile_via_ctypes
            mod.set_axon_ntff_profile_hook(
                _ntff_profile_via_ctypes("/opt/axon/libaxon_pjrt.so")
            )
        except Exception:
            pass


# ---------------------------------------------------------------------------
# Problem constants (hardcoded per contract)
# ---------------------------------------------------------------------------

B, S, DIN = 4, 2048, 1024
H, D = 16, 64
PROJ = H * D          # 1024
NCORES = 8
PL = PROJ // 2        # 512 per-core projection (8 heads)
HL = 8                # local heads
NPAIR = 4             # local head pairs
ST = S // 128         # 16 seq tiles (kpos chunks)
KT = DIN // 128       # 8 contraction tiles
VBLK = 256            # per (ktpair, head): 2 planes x [V(64)|ones|zeros(63)]
PAIRBLK = HL * VBLK   # 2048 cols per ktpair

# exp engine split: of every 16 chunks, this many go to ACT (exact exp),
# the rest to DVE (one-pass Schraudolph into fp8e4 bit patterns).
ACT_SHARE16 = int(os.environ.get("MHA_ACT_SHARE16", "8"))
SCHRAUD_C = float(os.environ.get("MHA_SCHRAUD_C", "-0.35"))
LOG2E = 1.4426950408889634

_cache = {}


def _build_program():
    import concourse.bass as bass
    import concourse.mybir as mybir
    import concourse.tile as tile

    f32 = mybir.dt.float32
    bf16 = mybir.dt.bfloat16
    f16 = mybir.dt.float16
    f8 = mybir.dt.float8e4
    i8 = mybir.dt.int8
    EXP = mybir.ActivationFunctionType.Exp
    IDENT = mybir.ActivationFunctionType.Identity
    DR = mybir.MatmulPerfMode.DoubleRow
    ADD = mybir.AluOpType.add
    SUB = mybir.AluOpType.subtract
    MULT = mybir.AluOpType.mult

    nc = bass.Bass("TRN2", target_bir_lowering=False, debug=False)

    x_d = nc.dram_tensor("x", [S, DIN], f16, kind="ExternalInput")
    wq_d = nc.dram_tensor("wq", [128, NPAIR * KT * 128], f16, kind="ExternalInput")
    wk_d = nc.dram_tensor("wk", [128, NPAIR * KT * 128], f16, kind="ExternalInput")
    wv_d = nc.dram_tensor("wv", [128, KT * PL], f16, kind="ExternalInput")
    bq_d = nc.dram_tensor("bq", [PL], f32, kind="ExternalInput")
    bvb_d = nc.dram_tensor("bvb", [128, PL], f16, kind="ExternalInput")
    wf_d = nc.dram_tensor("wf", [65, PL], f16, kind="ExternalInput")
    vse_d = nc.dram_tensor("vse", [64, HL], f32, kind="ExternalInput")
    vtz_d = nc.dram_tensor("vtz", [128, (ST // 2) * PAIRBLK], f8, kind="ExternalInput")
    y_d = nc.dram_tensor("y", [HL, D, S], f16, kind="ExternalOutput")
    s_d = nc.dram_tensor("s", [HL, S], f16, kind="ExternalOutput")

    with tile.TileContext(nc) as tc:
        with (
            tc.tile_pool(name="big", bufs=1) as big,
            tc.tile_pool(name="qk", bufs=2) as qkpool,
            tc.tile_pool(name="wblk", bufs=6) as wblk,
            tc.tile_pool(name="pt", bufs=6) as ptpool,
            tc.tile_pool(name="cx", bufs=2) as cxpool,
            tc.tile_pool(name="yst", bufs=2) as yst,
            tc.tile_pool(name="ps", bufs=1, space="PSUM") as ps,
            tc.tile_pool(name="ps2", bufs=4, space="PSUM") as ps2,
        ):
            # ---- persistent SBUF ------------------------------------------
            xt = big.tile([128, KT * S], f16, tag="xt")       # x^T, kt-blocked
            wv_sb = big.tile([128, KT * PL], f16, tag="wv")   # resident wv
            wq_sb = big.tile([128, NPAIR * KT * 128], f16, tag="wqs")
            wk_sb = big.tile([128, NPAIR * KT * 128], f16, tag="wks")
            vt = big.tile([128, (ST // 2) * PAIRBLK], f8, tag="vt")
            bqt = big.tile([128, NPAIR], f32, tag="bqt")
            bvt = big.tile([128, PL], f16, tag="bvt")
            wft = big.tile([65, PL], f16, tag="wft")
            vse = big.tile([64, HL], f32, tag="vse")
            ones8 = big.tile([128, 1], f8, tag="ones8")
            ident = big.tile([1, 1], f32, tag="ident")
            corrT = big.tile([65, HL], f32, tag="corrT")
            vs_sb = big.tile([1, PL], f32, tag="vs_sb")

            # Priority DMAs first: pair-0 weights + x^T transposes, so the
            # first projection chains start within a few us. The rest
            # streams in under the upfront compute block.
            sl0 = slice(0, KT * 128)
            nc.sync.dma_start(wq_sb[:, sl0], wq_d[:, sl0])
            nc.sync.dma_start(wk_sb[:, sl0], wk_d[:, sl0])
            nc.sync.dma_start(bqt[:], bq_d[:].rearrange("(t p) -> p t", p=128))
            for c in range(KT):
                nc.sync.dma_start_transpose(
                    xt[:, c * S:(c + 1) * S], x_d[:, c * 128:(c + 1) * 128]
                )
            nc.sync.dma_start(wv_sb[:], wv_d[:])
            nc.sync.dma_start(bvt[:], bvb_d[:])
            for c in range(1, 4):
                sl_ = slice(c * 1024, (c + 1) * 1024)
                nc.sync.dma_start(wq_sb[:, sl_], wq_d[:, sl_])
                nc.sync.dma_start(wk_sb[:, sl_], wk_d[:, sl_])
            nc.sync.dma_start(wft[:], wf_d[:])
            nc.sync.dma_start(vse[:], vse_d[:])

            nc.vector.memset(ones8[:], 1.0)
            nc.vector.memset(ident[:], 1.0)
            # vt ones/zeros scaffold pre-built on host
            nc.sync.dma_start(vt[:], vtz_d[:])

            # PSUM: sA/sB (ps2, [128,1024] x2bufs = 4 banks), c0 c1 (ctx),
            # p0 p1 (proj/fc/misc) = 8 banks total.

            def qt_tile():
                # [128 dq, 4 x (512 data + 512 zeros)] fp8
                return qkpool.tile([128, 2048], f16, tag="qt", name="qt")

            def kt_tile():
                # [128 dq, 2048 data + 128 zeros] fp8
                return qkpool.tile([128, 2048], f16, tag="ktr", name="ktr")

            # ---- QK projection (fp16, resident weights) ------------------
            def emit_qk_quanta(p, use_s=False):
                """Half-chain quanta (4 matmuls each; copy on 2nd half)."""
                qt_p = qt_tile()
                kt_p = kt_tile()
                quanta = []
                for w_sb, dst, is_q in ((wq_sb, qt_p, True), (wk_sb, kt_p, False)):
                    for ntg in range(2):
                        for i in range(2):
                            nt = ntg * 2 + i
                            for half in range(2):
                                def quantum(w_sb=w_sb, dst=dst, is_q=is_q,
                                            nt=nt, i=i, half=half):
                                    acc = ps.tile([128, 512], f32,
                                                  tag=f"p{i}", name="acc")
                                    for kk in range(4):
                                        kt = half * 4 + kk
                                        wof = (p * KT + kt) * 128
                                        nc.tensor.matmul(
                                            acc[:],
                                            w_sb[:, wof:wof + 128],
                                            xt[:, kt * S + nt * 512:
                                               kt * S + (nt + 1) * 512],
                                            start=(kt == 0), stop=(kt == KT - 1),
                                        )
                                    if half == 1:
                                        if is_q:
                                            nc.scalar.activation(
                                                dst[:, nt * 512:(nt + 1) * 512],
                                                acc[:], IDENT,
                                                bias=bqt[:, p:p + 1],
                                            )
                                        else:
                                            nc.scalar.copy(
                                                dst[:, nt * 512:(nt + 1) * 512],
                                                acc[:])
                                quanta.append(quantum)
                return (qt_p, kt_p), quanta

            # ---- V projection (bf16) -> vt fp8 ----------------------------
            def v_quantum(st):
                def quantum():
                    acc = ps.tile([128, PL], f32, tag="p0", name="acc")
                    for kt in range(KT):
                        nc.tensor.matmul(
                            acc[:],
                            xt[:, kt * S + st * 128: kt * S + (st + 1) * 128],
                            wv_sb[:, kt * PL:(kt + 1) * PL],
                            start=(kt == 0), stop=(kt == KT - 1),
                        )
                    t, i = st // 2, st % 2
                    dstv = vt[:, t * PAIRBLK:(t + 1) * PAIRBLK].rearrange(
                        "p (h a x) -> p h a x", a=2, x=128)[:, :, i, 0:64]
                    nc.vector.tensor_tensor(dstv, acc[:], bvt[:], op=ADD)
                return quantum

            # ---- V-sum correction chain -----------------------------------
            def vsum_quantum():
                def quantum():
                    vsp = ps.tile([1, PL], f32, tag="p0", name="vsp")
                    for st in range(ST):
                        t, i = st // 2, st % 2
                        mv = vt[:, t * PAIRBLK:(t + 1) * PAIRBLK].rearrange(
                            "p (h a x) -> p h a x", a=2, x=128)[:, :, i, 0:64]
                        nc.tensor.matmul(
                            vsp[:], ones8[:], mv,
                            start=(st == 0), stop=(st == ST - 1),
                        )
                    nc.vector.tensor_copy(vs_sb[:], vsp[:])
                return quantum

            def corr_quantum():
                def quantum():
                    v8t = ps.tile([64, HL], f32, tag="p0", name="v8t")
                    for h in range(HL):
                        nc.tensor.transpose(
                            v8t[:, h:h + 1],
                            vs_sb[0:1, h * 64:(h + 1) * 64],
                            ident[:],
                        )
                    nc.vector.tensor_tensor(
                        corrT[0:64, :], vse[:], v8t[:], op=SUB)
                    nc.vector.memset(corrT[64:65, :], 0.0)
                return quantum

            # ---- attention ------------------------------------------------
            chunk_no = [0]

            def emit_attention(p, qc, qt_p, kt_p, fill):
                """One (pair, qchunk): 2 heads x 8 ktpairs, then 2 FCs.

                ctx matmuls lag the scores by one kt-pair so the exp result
                they consume is always ready when the PE reaches them."""
                q0 = qc * 512
                ctxs = [ps.tile([128, 512], f32, tag="c0", name="ctx"),
                        ps.tile([128, 512], f32, tag="c1", name="ctx")]

                def do_ctx(h, t, pt_t):
                    gh = p * 2 + h
                    nc.tensor.matmul(
                        ctxs[h][:],
                        vt[:, t * PAIRBLK + gh * VBLK:
                           t * PAIRBLK + (gh + 1) * VBLK].rearrange(
                            "p (a x) -> p a x", x=128),
                        pt_t[:].rearrange("p (a x) -> p a x", a=2),
                        start=(t == 0), stop=(t == ST // 2 - 1),
                        perf_mode=DR,
                    )

                pend = []  # [(h, t, pt_t)] awaiting ctx matmuls
                for t in range(ST // 2):  # 8 kt pairs
                    pts = [ptpool.tile([128, 1024], f8, tag="pt", name="pt")
                           for _ in range(2)]
                    for i in range(2):
                        ki = 2 * t + i
                        # adjacent disjoint-row-group matmuls run
                        # concurrently in the PE array
                        for h in range(2):
                            r0 = h * 64
                            s_ps = ps2.tile([128, 512], f32, tag="s",
                                            name="s_ps")
                            nc.tensor.matmul(
                                s_ps[:],
                                kt_p[r0:r0 + 64, ki * 128:(ki + 1) * 128],
                                qt_p[r0:r0 + 64, q0:q0 + 512],
                                start=True, stop=True,
                                tile_position=(r0, 0),
                            )
                            cn = chunk_no[0]
                            chunk_no[0] += 1
                            # Bresenham interleave: ACT/DVE alternate
                            if ((cn + 1) * ACT_SHARE16) // 16 > \
                                    (cn * ACT_SHARE16) // 16:
                                nc.scalar.activation(
                                    pts[h][:, i * 512:(i + 1) * 512],
                                    s_ps[:], EXP, scale=0.125)
                            else:
                                nc.vector.tensor_scalar(
                                    pts[h][:, i * 512:(i + 1) * 512].bitcast(i8),
                                    s_ps[:],
                                    0.125 * 8.0 * LOG2E, 56.0 + SCHRAUD_C,
                                    op0=MULT, op1=ADD,
                                )
                            if fill and cn % 8 == 2:
                                fill.pop(0)()
                    while pend:
                        do_ctx(*pend.pop(0))
                    pend = [(0, t, pts[0]), (1, t, pts[1])]
                while pend:
                    do_ctx(*pend.pop(0))

                for h in range(2):
                    gh = p * 2 + h
                    ctx = ctxs[h]
                    cx = cxpool.tile([65, 512], f16, tag="cx", name="cx")
                    nc.scalar.activation(
                        cx[:], ctx[0:65, :], IDENT,
                        bias=corrT[:, gh:gh + 1])
                    nc.sync.dma_start(s_d[gh, qc * 512:(qc + 1) * 512],
                                      cx[64:65, :])
                    yp = ps.tile([64, 512], f32, tag=f"c{h}", name="yp")
                    nc.tensor.matmul(
                        yp[:], wft[:, gh * 64:(gh + 1) * 64], cx[:],
                        start=True, stop=True,
                    )
                    yo = yst.tile([64, 512], f16, tag="yo", name="yo")
                    nc.vector.tensor_copy(yo[:], yp[:])
                    nc.sync.dma_start(
                        y_d[gh, :, qc * 512:(qc + 1) * 512], yo[:])

            # ---- schedule -------------------------------------------------
            # upfront (dense PE block, warms the clock): qk pair 0, all of
            # V, the V-sum correction chain
            (qk_cur, quanta0) = emit_qk_quanta(0)
            for fn in quanta0:
                fn()
            for st in range(ST):
                v_quantum(st)()
            vsum_quantum()()
            corr_quantum()()

            fill = []
            qk_next = None
            for p in range(NPAIR):
                if p > 0:
                    qk_cur = qk_next
                for qc in range(4):
                    if qc == 0 and p + 1 < NPAIR:
                        (qk_next, quanta) = emit_qk_quanta(p + 1)
                        fill.extend(quanta)
                    emit_attention(p, qc, *qk_cur, fill)
            while fill:
                fill.pop(0)()

    return nc


def _prepare_in_maps(x, Wq, bq, Wk, bk, Wv, bv, Wf, bf):
    f16 = np.float16
    f8 = ml_dtypes.float8_e4m3
    in_maps = []
    x_16 = [np.ascontiguousarray(x[b]).astype(f16) for b in range(B)]

    # vt scaffold: zeros with 1.0 at col 64 of every 128-block
    vtz = np.zeros((128, (ST // 2) * PAIRBLK), dtype=f8)
    vtz.reshape(128, -1, 128)[:, :, 64] = f8(1.0)

    for core in range(NCORES):
        b, g = core // 2, core % 2
        sl = slice(g * PL, (g + 1) * PL)

        def _tile_w(w):  # [1024, 512] -> [128, (pair, kt, 128)]
            return np.ascontiguousarray(
                w.reshape(KT, 128, NPAIR, 128).transpose(1, 2, 0, 3).reshape(
                    128, NPAIR * KT * 128)
            ).astype(f16)

        wv_g = np.ascontiguousarray(Wv[:, sl]).astype(f16)
        # exact col-sums of V (incl bias) for the correction, [64, HL]
        v_exact = x_16[b].astype(np.float32) @ wv_g.astype(np.float32) \
            + bv[sl].astype(np.float32)
        vse = v_exact.sum(axis=0, dtype=np.float64).astype(np.float32)
        vse = np.ascontiguousarray(vse.reshape(HL, 64).T)  # [64, HL]

        wf_s = np.zeros((65, PL), dtype=np.float16)
        for h in range(HL):
            wf_s[0:64, h * 64:(h + 1) * 64] = \
                Wf[g * PL + h * 64: g * PL + (h + 1) * 64, :]

        in_maps.append({
            "x": x_16[b],
            "wq": _tile_w(Wq[:, sl]),
            "wk": _tile_w(Wk[:, sl]),
            "wv": np.ascontiguousarray(
                wv_g.reshape(KT, 128, PL).transpose(1, 0, 2).reshape(
                    128, KT * PL)),
            "bq": np.ascontiguousarray(bq[sl]).astype(np.float32),
            "bvb": np.broadcast_to(bv[sl], (128, PL)).astype(f16).copy(),
            "wf": wf_s,
            "vse": vse,
            "vtz": vtz,
        })
    return in_maps


def kernel(**inputs):
    _ensure_patches()
    _ensure_profile_hook()
    from concourse.bass_utils import run_bass_kernel_spmd

    if "nc" not in _cache:
        _cache["nc"] = _build_program()
    nc = _cache["nc"]

    inp = {k: np.asarray(v, dtype=np.float32) for k, v in inputs.items()}
    in_maps = _prepare_in_maps(**inp)

    trace = bool(os.environ.get("MHA_TRACE"))
    res = run_bass_kernel_spmd(nc, in_maps, list(range(NCORES)), trace=trace)
    _cache["last_results"] = res

    bf = inp["bf"]
    out = np.empty((B, S, D), dtype=np.float32)
    for b in range(B):
        acc = np.zeros((D, S), dtype=np.float64)
        for core in (2 * b, 2 * b + 1):
            yc = np.asarray(res.results[core]["y"]).astype(np.float64)
            sc = np.asarray(res.results[core]["s"]).astype(np.float64)
            acc += (yc / sc[:, None, :]).sum(axis=0)
        out[b] = acc.T + bf
    return out
